# revision 9
# baseline (speedup 1.0000x reference)
"""Multi-head attention (B=2, N=2048, dim=1024, heads=16, dim_head=64) on
8 TRN2 NeuronCores.

Sharding: data-parallel over batch (2) x tensor-parallel over heads (4 per
core).  Core c handles batch b = c//4 and heads [4g, 4g+4), g = c%4.  Each
core computes its 4 heads' attention plus the partial out-projection; the
host sums the 4 partials per batch and adds the bias.

Pipeline design (vs. the previous version):
  * Streaming prologue: xt arrives chunk-by-chunk over 3 DMA launcher
    engines (sync/gpsimd/scalar); Kt/V tiles are produced just-in-time as
    fillers inside the attention loop, so the first exp fires at ~10us
    instead of ~40us.
  * exp batching: St for an mt-PAIR lands in one [128, 1024] 2-bank PSUM
    tile; one ACTIVATE covers both tiles -> ScalarE overhead per element
    drops ~20%.
  * AV lags St by 2 mt-pairs so filler DMA waits never starve ScalarE.
  * Output staged in fp16 (half the writeback traffic); host accumulates
    partials in fp32.

Per-iteration steady state (one mt-pair, one head pair, one 512-query
chunk): PE: 4 AV MMs (lag 2) + 4 St MMs + fillers; ScalarE: 2 x
exp([128,1024]); DVE/GpSimd: evacuations + normalize.  PSUM: st 2x
[128,1024] (4 banks) + ot 2x[65,512] (2) + qk 2x[128,512] (2) = 8 banks.
"""
import numpy as np

import concourse.bass as bass
import concourse.mybir as mybir
import concourse.tile as tile
from concourse import bacc
from concourse.bass_utils import run_bass_kernel_spmd

# Problem constants (hardcoded per contract).
B = 2
N = 2048
DIM = 1024
HEADS = 16
DH = 64
INNER = HEADS * DH
SCALE = DH ** -0.5

N_CORES = 8
HEADS_PER_CORE = 4
PAIRS = 2          # head pairs per core
NT = N // 128      # 16 key tiles
DT = DIM // 128    # 8 contraction tiles
CH = N // 512      # 4 query chunks
MTP = NT // 2      # 8 key-tile pairs per chunk
F32 = mybir.dt.float32
F16 = mybir.dt.float16
BF16 = mybir.dt.bfloat16

# wqkv column layout (host-packed): [k_p0 | q_p0 | v | k_p1 | q_p1]
KCOL = {0: 0, 1: 512}
QCOL = {0: 128, 1: 640}
VCOL = 256

_CACHED_NC = None


def _emit_kernel(tc, xt_d, wqkv_d, wo_d, out_d):
    nc = tc.nc

    from contextlib import ExitStack

    ctx = ExitStack()
    per = ctx.enter_context(tc.tile_pool(name="persist", bufs=1))
    psum = ctx.enter_context(tc.tile_pool(name="psum", bufs=1, space="PSUM"))
    work = ctx.enter_context(tc.tile_pool(name="work", bufs=1))

    # Persistent SBUF tensors.
    xt_sb = per.tile([128, DT, N], BF16, tag="xt")
    wqkv_sb = per.tile([128, DT, 768], BF16, tag="wqkv")
    wo_sb = per.tile([128, PAIRS, DIM], BF16, tag="wo")
    qt_sb = per.tile([128, PAIRS, N], BF16, tag="qt")
    kt_sb = per.tile([128, PAIRS, N], BF16, tag="kt")
    v_sb = per.tile([128, NT, HEADS_PER_CORE, DH + 1], BF16, tag="v")
    o_sb = per.tile([128, PAIRS, N], BF16, tag="o")

    # ---- Input DMAs, split across launcher engines so the first chunk of
    # xt plus the pair-0 QK weights land in ~6us.  Each launch is one
    # descriptor chain on one HW queue; sync issues the early-needed data.
    # wqkv_d is host-packed in SBUF layout [128, DT, 768] (flattened to
    # [128, 6144]) so every DMA below is a 1:1 strided slice copy.
    wqkv_src = wqkv_d.rearrange("p (d c) -> p d c", c=768)
    for t in range(4):       # k_p0 | q_p0 weight block, 2 dt-tiles per DMA
        nc.sync.dma_start(
            wqkv_sb[:, 2 * t:2 * t + 2, 0:256], wqkv_src[:, 2 * t:2 * t + 2, 0:256]
        )
    for dt in range(DT):     # xt chunk 0
        nc.sync.dma_start(xt_sb[:, dt, 0:512], xt_d[128 * dt:128 * (dt + 1), 0:512])
    for t in range(4):       # v weights
        nc.sync.dma_start(
            wqkv_sb[:, 2 * t:2 * t + 2, 256:512],
            wqkv_src[:, 2 * t:2 * t + 2, 256:512],
        )
    for dt in range(DT):     # xt chunk 1 (gpsimd launcher)
        nc.gpsimd.dma_start(
            xt_sb[:, dt, 512:1024], xt_d[128 * dt:128 * (dt + 1), 512:1024]
        )
    for t in range(2):       # k_p1 | q_p1 weights
        nc.gpsimd.dma_start(
            wqkv_sb[:, 4 * t:4 * t + 4, 512:768],
            wqkv_src[:, 4 * t:4 * t + 4, 512:768],
        )
    for p in range(PAIRS):   # out-projection weights
        nc.gpsimd.dma_start(wo_sb[:, p, :], wo_d[128 * p:128 * (p + 1), :])
    for dt in range(DT):     # xt chunk 2 (scalar launcher, idle until exps)
        nc.scalar.dma_start(
            xt_sb[:, dt, 1024:1536], xt_d[128 * dt:128 * (dt + 1), 1024:1536]
        )
    for dt in range(DT):     # xt chunk 3
        nc.sync.dma_start(
            xt_sb[:, dt, 1536:2048], xt_d[128 * dt:128 * (dt + 1), 1536:2048]
        )

    # Ones column of V' (gives the softmax denominator through the AV matmul).
    ones_sb = per.tile([128, NT * HEADS_PER_CORE], F32, tag="ones")
    nc.vector.memset(ones_sb[:], 1.0)
    nc.vector.tensor_copy(
        v_sb[:, :, :, DH:DH + 1],
        ones_sb[:].rearrange("p (a b c) -> p a b c", b=HEADS_PER_CORE, c=1),
    )
    # Touch Exp once so the ACT table DMA (~2.7us) happens during startup.
    warm = work.tile([1, 1], F32, tag="warm")
    nc.scalar.activation(
        warm[:], ones_sb[0:1, 0:1], mybir.ActivationFunctionType.Exp, scale=1.0
    )
    # Warm the HAM clock gate while input DMAs land: ~6us of dummy matmuls
    # (fp32: 4 cycles/row) trip the fully-busy window so real work starts
    # at 2.4 GHz.
    for i in range(28):
        dummy = psum.tile([64, 64], F32, tag="qk", bufs=2, name="dummy")
        nc.tensor.matmul(
            dummy[:], ones_sb[:, 0:64], ones_sb[:, 0:64], start=True, stop=True
        )

    def emit_qk_chunk(which, p, c):
        """Qt or Kt for head pair p, n-chunk c: [128, 512] of W.T @ xT."""
        src = qt_sb if which == "q" else kt_sb
        col0 = (QCOL if which == "q" else KCOL)[p]
        ps = psum.tile([128, 512], F32, tag="qk", bufs=2)
        for dt in range(DT):
            nc.tensor.matmul(
                ps[:],
                wqkv_sb[:, dt, col0:col0 + 128],
                xt_sb[:, dt, 512 * c:512 * (c + 1)],
                start=(dt == 0),
                stop=(dt == DT - 1),
            )
        nc.vector.tensor_copy(src[:, p, 512 * c:512 * (c + 1)], ps[:])

    def emit_v_tile(mt):
        """V natural [128(m), 256(4 heads x 64)] for key tile mt."""
        ps = psum.tile([128, 256], F32, tag="qk", bufs=2)
        for dt in range(DT):
            nc.tensor.matmul(
                ps[:],
                xt_sb[:, dt, 128 * mt:128 * (mt + 1)],
                wqkv_sb[:, dt, VCOL:VCOL + 256],
                start=(dt == 0),
                stop=(dt == DT - 1),
            )
        nc.vector.tensor_copy(
            v_sb[:, mt, :, 0:DH],
            ps[:].rearrange("p (h d) -> p h d", h=HEADS_PER_CORE),
        )

    ev_tiles = {}

    def emit_proj_unit(nt, jc, evac=None):
        """out[128nt:+128, 512jc:+512] = sum_p o_sb[:,p,nt].T @ wo[:,p,jc]."""
        if nt not in ev_tiles:
            ev_tiles[nt] = work.tile([128, DIM], F16, tag="ev", bufs=3, name="ev")
        ev = ev_tiles[nt]
        ps = psum.tile([128, 512], F32, tag="qk", bufs=2)
        for p in range(PAIRS):
            nc.tensor.matmul(
                ps[:],
                o_sb[:, p, 128 * nt:128 * (nt + 1)],
                wo_sb[:, p, 512 * jc:512 * (jc + 1)],
                start=(p == 0),
                stop=(p == PAIRS - 1),
            )
        if evac == "scalar":
            nc.scalar.copy(ev[:, 512 * jc:512 * (jc + 1)], ps[:])
        else:
            nc.vector.tensor_copy(ev[:, 512 * jc:512 * (jc + 1)], ps[:])
        if jc == 1:
            nc.sync.dma_start(out_d[128 * nt:128 * (nt + 1), :], ev[:])
            del ev_tiles[nt]

    def run_filler(unit):
        kind = unit[0]
        if kind == "v":
            emit_v_tile(unit[1])
        elif kind == "qk":
            emit_qk_chunk(unit[1], unit[2], unit[3])
        else:
            emit_proj_unit(unit[1], unit[2], evac=unit[3])

    def emit_att_chunk(p, c, fillers):
        """Attention for head pair p, query chunk c (cols 512c..512c+512).

        fillers: dict iter->list of units emitted after that iteration's
        exps (their DMA waits then eat ScalarE slack, not the St path).
        AV for mt-pair k is flushed at iteration k+2.
        """
        ot = [
            psum.tile([DH + 1, 512], F32, tag="ot", bufs=2, name=f"ot{h}")
            for h in range(2)
        ]
        pending = []

        def flush_av(n):
            for pmtp, ppts in pending[:n]:
                for h in range(2):
                    for j in range(2):
                        mt = 2 * pmtp + j
                        nc.tensor.matmul(
                            ot[h][:],
                            v_sb[:, mt, 2 * p + h, :],
                            ppts[h][:, 512 * j:512 * (j + 1)],
                            start=(mt == 0),
                            stop=(mt == NT - 1),
                        )
            del pending[:n]

        for mtp in range(MTP):
            if len(pending) == 2:
                flush_av(1)
            st = [None, None]
            for h in range(2):
                st[h] = psum.tile([128, 1024], F32, tag="st", bufs=2, name=f"st{h}")
                for j in range(2):
                    mt = 2 * mtp + j
                    nc.tensor.matmul(
                        st[h][:, 512 * j:512 * (j + 1)],
                        kt_sb[64 * h:64 * (h + 1), p, 128 * mt:128 * (mt + 1)],
                        qt_sb[64 * h:64 * (h + 1), p, 512 * c:512 * (c + 1)],
                        start=True,
                        stop=True,
                        tile_position=(64 * h, 0),
                    )
            pts = [None, None]
            for h in range(2):
                pts[h] = work.tile([128, 1024], BF16, tag="pt", bufs=8, name=f"pt{h}")
                for j in range(2):
                    nc.scalar.activation(
                        pts[h][:, 512 * j:512 * (j + 1)],
                        st[h][:, 512 * j:512 * (j + 1)],
                        mybir.ActivationFunctionType.Exp,
                        scale=SCALE,
                    )
            pending.append((mtp, pts))
            for unit in fillers.get(mtp, ()):
                run_filler(unit)
        flush_av(len(pending))
        for unit in fillers.get(MTP, ()):
            run_filler(unit)
        # Normalize: o = Ot'[0:64] / Ot'[64], interleaved across heads.
        den, recip, rbc = [None, None], [None, None], [None, None]
        for h in range(2):
            den[h] = work.tile([1, 512], F32, tag="den", bufs=4, name=f"den{h}")
            nc.vector.tensor_copy(den[h][:], ot[h][DH:DH + 1, :])
        for h in range(2):
            recip[h] = work.tile([1, 512], F32, tag="recip", bufs=4, name=f"rec{h}")
            nc.vector.reciprocal_approx_fast(recip[h][:], den[h][:])
        for h in range(2):
            rbc[h] = work.tile([64, 512], F32, tag="rbc", bufs=4, name=f"rbc{h}")
            nc.gpsimd.partition_broadcast(rbc[h][:], recip[h][:])
        for h in range(2):
            nc.vector.tensor_mul(
                o_sb[64 * h:64 * (h + 1), p, 512 * c:512 * (c + 1)],
                ot[h][0:DH, :],
                rbc[h][:],
            )

    # ---- Emission schedule ----
    # Prologue: just enough for the first St + the first AV flushes.
    emit_qk_chunk("k", 0, 0)
    emit_qk_chunk("q", 0, 0)
    emit_v_tile(0)
    emit_v_tile(1)

    # Pair 0.  V tiles and Kt chunks stream in ~2 iterations ahead of their
    # consumers; xt chunk k lands at roughly 7+4k us.
    p0_fill = {
        0: {0: [("v", 2)], 1: [("v", 3), ("qk", "k", 0, 1)],
            2: [("v", 4), ("v", 5)], 3: [("qk", "k", 0, 2), ("v", 6), ("v", 7)],
            4: [("v", 8), ("v", 9)], 5: [("qk", "k", 0, 3), ("v", 10), ("v", 11)],
            6: [("v", 12), ("v", 13)], 7: [("v", 14), ("v", 15)],
            8: [("qk", "q", 0, 1)]},
        1: {0: [("qk", "k", 1, 0)], 2: [("qk", "k", 1, 1)],
            4: [("qk", "k", 1, 2)], 6: [("qk", "k", 1, 3)],
            8: [("qk", "q", 0, 2)]},
        2: {1: [("qk", "q", 1, 0)], 5: [("qk", "q", 0, 3)]},
        3: {1: [("qk", "q", 1, 1)], 5: [("qk", "q", 1, 2)]},
    }
    for c in range(CH):
        emit_att_chunk(0, c, p0_fill[c])

    # Pair 1.  Chunk c's projection units run as fillers inside chunk c+1.
    def proj_units(nts, evac=None):
        return [("proj", nt, jc, evac) for nt in nts for jc in range(2)]

    p1_fill = {
        0: {1: [("qk", "q", 1, 3)]},
        1: {m: [u] for m, u in enumerate(proj_units(range(0, 4)))},
        2: {m: [u] for m, u in enumerate(proj_units(range(4, 8)))},
        3: {m: [u] for m, u in enumerate(proj_units(range(8, 12)))},
    }
    for c in range(CH):
        emit_att_chunk(1, c, p1_fill[c])
    # Tail drain: remaining projections with evacuations spread across
    # ScalarE / DVE / GpSimd (ScalarE is idle now).
    evacs = ["scalar", None, "scalar", None, "scalar", None, "scalar", None]
    for i, (nt, jc) in enumerate([(nt, jc) for nt in range(12, 16) for jc in range(2)]):
        emit_proj_unit(nt, jc, evac=evacs[i])

    ctx.close()


def _build():
    global _CACHED_NC
    if _CACHED_NC is not None:
        return _CACHED_NC
    nc = bacc.Bacc(
        "TRN2",
        target_bir_lowering=False,
        debug=False,
        enable_asserts=True,
        num_devices=N_CORES,
    )
    xt_d = nc.dram_tensor("xt", [DIM, N], BF16, kind="ExternalInput").ap()
    wqkv_d = nc.dram_tensor("wqkv", [128, DT * 768], BF16, kind="ExternalInput").ap()
    wo_d = nc.dram_tensor("wo", [256, DIM], BF16, kind="ExternalInput").ap()
    out_d = nc.dram_tensor("out", [N, DIM], F16, kind="ExternalOutput").ap()

    with tile.TileContext(nc) as tc:
        _emit_kernel(tc, xt_d, wqkv_d, wo_d, out_d)
    nc.compile()
    _CACHED_NC = nc
    return nc


def _in_maps(x, w_qkv, w_out):
    import ml_dtypes

    bf = ml_dtypes.bfloat16
    maps = []
    for c in range(N_CORES):
        b, g = divmod(c, 4)
        cols = slice(256 * g, 256 * (g + 1))
        q = w_qkv[:, cols]
        k = w_qkv[:, INNER:][:, cols]
        v = w_qkv[:, 2 * INNER:][:, cols]
        # Column layout [k_p0 | q_p0 | v | k_p1 | q_p1] so the earliest-
        # needed weights are first in DMA order; then repacked into the
        # SBUF tile layout [128, DT, 768] so device DMAs are 1:1 slices.
        wqkv_c = np.concatenate(
            [k[:, 0:128], q[:, 0:128], v, k[:, 128:256], q[:, 128:256]],
            axis=1,
        )
        wqkv_c = np.ascontiguousarray(
            wqkv_c.reshape(DT, 128, 768).transpose(1, 0, 2).reshape(128, DT * 768)
            .astype(bf)
        )
        maps.append(
            {
                "xt": np.ascontiguousarray(x[b].T.astype(bf)),
                "wqkv": wqkv_c,
                "wo": np.ascontiguousarray(w_out[cols, :].astype(bf)),
            }
        )
    return maps


def _run(x, w_qkv, w_out, b_out, trace=False):
    nc = _build()
    res = run_bass_kernel_spmd(
        nc, _in_maps(x, w_qkv, w_out), list(range(N_CORES)), trace=trace
    )
    partials = np.stack(
        [res.results[c]["out"].astype(np.float32) for c in range(N_CORES)]
    )
    out = np.empty((B, N, DIM), dtype=np.float32)
    for b in range(B):
        out[b] = partials[4 * b:4 * b + 4].sum(axis=0) + b_out
    return out, res


def kernel(x, w_qkv, w_out, b_out):
    out, _ = _run(
        np.asarray(x, dtype=np.float32),
        np.asarray(w_qkv, dtype=np.float32),
        np.asarray(w_out, dtype=np.float32),
        np.asarray(b_out, dtype=np.float32),
    )
    return out
